# revision 23
# baseline (speedup 1.0000x reference)
"""Multi-head attention (GAttention) on 8 trn2 NeuronCores — v2.

Reference computation (per batch b):
    q = x @ w_qkv.T            -> [N, 768], heads of 64
    attn = softmax(q k^T / 8)  -> per head [N, M]
    out_h = attn @ v           -> [N, 64]
    out = concat(out_h) @ w_proj.T + b_proj

Sharding: 24 (b, head) units over 8 cores -> each core one batch b and 3
heads; cores emit f16 partial projections [N, 768]; host sums 4 partials
per batch + bias.

v2 design vs the 190us baseline (all engines rebalanced around the serial
exp softmax cost):
  * exp is split between the Scalar (ACT) engine and a CUSTOM DVE op
    "EXP32Q_ANT": e^x ~ ((t+A)t+B)^32 with t = sigma-scaled logits, a
    single 8-stage Vector-engine instruction (quadratic Horner + 5
    squarings).  sigma = 0.125/(32 kk) is folded into w_q host-side so
    S^T arrives pre-scaled; the ACT path uses activation scale 32 kk.
  * one flat phase: qproj h0 runs first (chunked, 512 cols at a time so
    attention starts ~6us in); qproj h1/h2 and the projection of each
    completed n-quarter are interleaved into the attention iteration
    stream, so the PE never waits on phase barriers.
  * PSUM (8 banks): st 2x[128,2,512] (4) + av 2x[128,512] (2) + one
    shared [128,512]/[128,2,384] ring (2) used by qproj early and proj
    later.
  * proj packs n-tile pairs (rows 0:64 / 64:128 of the PE) into one
    [128,2,384] PSUM tile; a single engine copy casts to f16 and one
    strided DMA writes both n-tiles.
  * outTn row duplication (needed for proj row packing) is done by a
    SBUF->SBUF DMA instead of a second vector multiply.
  * inputs are host-packed partition-major so every DMA is a few large
    contiguous-row transfers (descriptor spreading across all 16 DMA
    engines does the balancing); issue order follows consumption order.
"""
import numpy as np
import ml_dtypes
from contextlib import ExitStack

import concourse.bass as bass
import concourse.mybir as mybir
import concourse.tile as tile
from concourse import bacc
from concourse.bass_utils import run_bass_kernel_spmd

B, N, DIM = 2, 2048, 768
H, D = 12, 64
M = 2048
NCORES = 8
HPC = 3              # heads per core
NT = N // 128        # 16 n tiles
MT = M // 128        # 16 m tiles
MP = MT // 2         # 8 m-tile pairs
CT = DIM // 128      # 6 contraction tiles for qproj
NQ = 4               # n quarters of 512
QW = N // NQ         # 512
F32 = mybir.dt.float32
F16 = mybir.dt.float16
BF16 = mybir.dt.bfloat16

# --- custom DVE exp: e^x ~ ((t + A)*t + B)^32, t = x/(32*KK) -------------
# minimax fit weighted by softmax contribution (see fit, rel err <=1e-3 on
# the weight-bearing logit range; end-to-end rel err 4.9e-3 all-DVE).
KK = 1.33947417
EA = 1.33152807
EB = 1.00021259
ACT_SCALE = 32.0 * KK            # ACT path: e^{st * ACT_SCALE}
SIGMA = 0.125 / (32.0 * KK)      # folded into w_q host-side

import os
DVE_PER16 = int(os.environ.get("KX", "5"))   # exp tiles per 16 on DVE
_DVE_MOD = set(round(i * 16 / max(DVE_PER16, 1) + 1) % 16
               for i in range(DVE_PER16))
LAG = 2

_cached = {}


def _register_exp_op():
    import concourse.dve_ops as dvo
    from concourse.dve_spec import Spec, Src0, C0, C1, sq, lower
    from concourse.dve_uop import DveOpSpec

    name = "EXP32Q_ANT"
    for op in dvo.OPS:
        if op.name == name:
            return op
    p = (Src0 + C0) * Src0 + C1
    for _ in range(5):
        p = sq(p)
    spec = Spec(body=p)
    row = max(dvo._SUB_OPCODE_FOR_NAME.values()) + 1
    assert row < 0x20
    dvo._SUB_OPCODE_FOR_NAME[name] = row
    shas = {}
    for ver in ("v3", "v4"):
        try:
            uops = lower(spec, ver=ver)
            shas[ver] = DveOpSpec(
                name=name, opcode=row, uops=uops, rd1_en=False).sha(ver)
        except Exception:
            pass
    op = dvo.DveOp(name, spec, subdim=False, uops_sha=shas)
    dvo.OPS.append(op)
    dvo.CUSTOM_DVE_SPECS[name] = spec
    return op


def build_program():
    exp_op = _register_exp_op()
    nc = bacc.Bacc("TRN2", target_bir_lowering=False, debug=False)
    xT_d = nc.dram_tensor("xT", [DIM, N], BF16, kind="ExternalInput")
    wq_d = nc.dram_tensor("wq", [128, HPC, CT, 128], BF16,
                          kind="ExternalInput")
    kT_d = nc.dram_tensor("kT", [128, HPC, MP, 128], BF16,
                          kind="ExternalInput")
    va_d = nc.dram_tensor("va", [128, HPC, MT, 128], BF16,
                          kind="ExternalInput")
    wp_d = nc.dram_tensor("wp", [128, HPC, DIM], BF16, kind="ExternalInput")
    out_d = nc.dram_tensor("out", [N, DIM], F32, kind="ExternalOutput")

    with tile.TileContext(nc) as tc, ExitStack() as ctx:
        big = ctx.enter_context(tc.tile_pool(name="big", bufs=1))
        etp = ctx.enter_context(tc.tile_pool(name="etp", bufs=6))
        rsp = ctx.enter_context(tc.tile_pool(name="rsp", bufs=2))
        stg = ctx.enter_context(tc.tile_pool(name="stg", bufs=3))

        # persistent SBUF tensors; DMA issue order == consumption order,
        # spread across engine queues (each dma_start costs ~600ns of issue
        # time on its sequencer, so the head-of-kernel loads go wide).
        wq_t = big.tile([128, HPC, CT, 128], BF16)
        nc.scalar.dma_start(wq_t[:], wq_d[:])
        xT_t = [big.tile([128, N], BF16, name=f"xT{c}", tag=f"xT{c}")
                for c in range(CT)]
        for c in range(CT):
            eng = nc.sync if c % 2 == 0 else nc.scalar
            eng.dma_start(xT_t[c][:, 0:QW],
                          xT_d[c * 128:(c + 1) * 128, 0:QW])
        kT_t = big.tile([128, HPC, MP, 128], BF16)
        nc.sync.dma_start(kT_t[:], kT_d[:])
        va_t = big.tile([128, HPC, MT, 128], BF16)
        nc.sync.dma_start(va_t[:, :, 0:4, :], va_d[:, :, 0:4, :])
        for c in range(CT):
            nc.sync.dma_start(xT_t[c][:, QW:N],
                              xT_d[c * 128:(c + 1) * 128, QW:N])
        nc.sync.dma_start(va_t[:, :, 4:MT, :], va_d[:, :, 4:MT, :])
        wp_t = big.tile([128, HPC, DIM], BF16)
        nc.sync.dma_start(wp_t[:], wp_d[:])

        qT_t = big.tile([128, HPC, N], BF16)
        outTn_t = big.tile([128, HPC, N], BF16)
        zz_t = big.tile([128, 64], BF16)
        nc.vector.memset(zz_t[:], 0.0)

        st_ps = ctx.enter_context(
            tc.tile_pool(name="st_ps", bufs=2, space="PSUM"))
        acc_ps = ctx.enter_context(
            tc.tile_pool(name="acc_ps", bufs=2, space="PSUM"))

        def qproj_chunk(h, ch):
            # q columns [ch*512, (ch+1)*512) for head h (duplicated rows)
            qp = acc_ps.tile([128, 2, QW], F32, tag="acc", name="acc",
                             bufs=1)
            nsl = slice(ch * QW, (ch + 1) * QW)
            for c in range(CT):
                nc.tensor.matmul(
                    qp[:, 0, :], wq_t[:, h, c, :], xT_t[c][:, nsl],
                    start=(c == 0), stop=(c == CT - 1),
                )
            nc.vector.tensor_copy(qT_t[:, h, nsl], qp[:, 0, :])

        def proj_group(q, j, oc):
            # n-tile pair (a=rows 0:64 lhs, b=rows 64:128), output column
            # slice oc*384; one packed PSUM tile, one f16 copy, one DMA.
            na = (q * 4 + 2 * j) * 128
            nb = na + 128
            osl = slice(oc * 384, (oc + 1) * 384)
            pj = acc_ps.tile([128, 2, QW], F32, tag="acc", name="acc",
                             bufs=1)
            pjv = pj[:, :, 0:384]
            for hh in range(HPC):
                nc.tensor.matmul(
                    pjv[:, 0, :], outTn_t[0:64, hh, na:na + 128],
                    wp_t[0:64, hh, osl],
                    start=(hh == 0), stop=(hh == HPC - 1),
                    tile_position=(0, 0),
                )
                nc.tensor.matmul(
                    pjv[:, 1, :], outTn_t[64:128, hh, nb:nb + 128],
                    wp_t[64:128, hh, osl],
                    start=(hh == 0), stop=(hh == HPC - 1),
                    tile_position=(64, 0),
                )
            ot = stg.tile([128, 2, 384], F32, tag="ot", name="ot")
            if (j + oc) % 2 == 0:
                nc.scalar.copy(ot[:], pjv[:])
            else:
                nc.vector.tensor_copy(ot[:], pjv[:])
            nc.sync.dma_start(out_d[na:na + 128, osl], ot[:, 0, :])
            nc.sync.dma_start(out_d[nb:nb + 128, osl], ot[:, 1, :])

        # attention iteration stream with injected qproj/proj work; unit
        # (0,h1) starts at gi 8 and quarter 1 at gi 24, so chunk (h,ch) is
        # always emitted well before its first reader
        units = [(q, h) for q in range(NQ) for h in range(HPC)]
        inject = {}
        for ch in range(1, 4):
            inject[3 * ch - 2] = [("qproj", 0, ch)]
        for ch in range(4):
            inject[3 * ch + 2] = [("qproj", 1, ch)]
            inject.setdefault(3 * ch + 3, []).append(("qproj", 2, ch))
        # proj for quarter q injects only after unit (q, h2)'s lagged AV
        # flush + norm have been EMITTED (at iter base+1's flush), else the
        # dep tracker can't order the proj reads after the norm writes.
        for q in range(NQ - 1):
            base = (3 * q + 3) * 8 + 1
            for g in range(4):
                inject.setdefault(base + 2 * g, []).append(
                    ("proj", q, g // 2, g % 2))

        av_by_unit = {}
        pend = []

        def _av(pd):
            (q, h), et, p, first, last, unit_last = pd
            av = av_by_unit[(q, h)]
            nc.tensor.matmul(av[:], va_t[:, h, 2 * p, :], et[:, 0, :],
                             start=first, stop=False)
            nc.tensor.matmul(av[:], va_t[:, h, 2 * p + 1, :], et[:, 1, :],
                             start=False, stop=last)

        def _norm(q, h):
            av = av_by_unit[(q, h)]
            nsl = slice(q * QW, (q + 1) * QW)
            rs = rsp.tile([64, QW], F32, tag="rs", name="rs")
            # va cols 0:64 are ones -> av[0:64] = denominator (base-0 AP:
            # custom-DVE ops misread partition-base-64 inputs)
            nc.vector.reciprocal_approx_fast(rs[:], av[0:64, :])
            nc.vector.tensor_mul(outTn_t[0:64, h, nsl], av[64:128, :], rs[:])
            nc.sync.dma_start(outTn_t[64:128, h, nsl],
                              outTn_t[0:64, h, nsl])

        def _flush(limit):
            while len(pend) > limit:
                pd = pend.pop(0)
                _av(pd)
                if pd[5]:
                    _norm(*pd[0])

        # PE p-state warmup: dummy matmuls on a zeroed tile into scrap PSUM
        # keep the array busy while the first DMAs land
        for w in range(2):
            wt = st_ps.tile([128, 2, QW], F32, tag="st", name="st")
            for i in range(8):
                nc.tensor.matmul(wt[0:64, 0, 0:64], zz_t[:, 0:64],
                                 zz_t[:, 0:64], start=True, stop=True)

        qproj_chunk(0, 0)

        gi = 0
        for (q, h) in units:
            nsl = slice(q * QW, (q + 1) * QW)
            for p in range(MP):
                if p == 0:
                    av_by_unit[(q, h)] = acc_ps.tile(
                        [128, QW], F32, tag="av", name="av")
                st = st_ps.tile([128, 2, QW], F32, tag="st", name="st")
                nc.tensor.matmul(
                    st[:, 0, :], kT_t[0:64, h, p, :], qT_t[0:64, h, nsl],
                    start=True, stop=True, tile_position=(0, 0),
                )
                nc.tensor.matmul(
                    st[:, 1, :], kT_t[64:128, h, p, :],
                    qT_t[64:128, h, nsl],
                    start=True, stop=True, tile_position=(64, 0),
                )
                _flush(LAG - 1)
                et = etp.tile([128, 2, QW], BF16, tag="et", name="et")
                if (gi % 16) in _DVE_MOD:
                    nc.vector._custom_dve(
                        exp_op, out=et[:], in0=st[:],
                        s0=float(EA), s1=float(EB))
                else:
                    nc.scalar.activation(
                        et[:], st[:], mybir.ActivationFunctionType.Exp,
                        scale=ACT_SCALE)
                pend.append(((q, h), et, p, p == 0, p == MP - 1,
                             p == MP - 1))
                for item in inject.get(gi, ()):
                    if item[0] == "qproj":
                        qproj_chunk(item[1], item[2])
                    else:
                        proj_group(item[1], item[2], item[3])
                gi += 1
        _flush(0)
        for g in range(4):
            proj_group(NQ - 1, g // 2, g % 2)

    nc.compile()
    return nc


def build_in_maps(x, k, v, w_qkv, w_proj):
    x = np.asarray(x, dtype=np.float32)
    k = np.asarray(k, dtype=np.float32)
    v = np.asarray(v, dtype=np.float32)
    wqT = np.ascontiguousarray(np.asarray(w_qkv, np.float32).T) * SIGMA
    wpT = np.ascontiguousarray(np.asarray(w_proj, np.float32).T)
    bf = ml_dtypes.bfloat16

    in_maps = []
    for core in range(NCORES):
        b = core // 4
        hs = [3 * (core % 4) + i for i in range(HPC)]
        xT = np.ascontiguousarray(x[b].T.astype(bf))
        # wq packed [128, HPC, CT, 128]: partition = contraction row within
        # c-tile; last dim = duplicated 64-wide head slice
        wq = np.empty((128, HPC, CT, 128), dtype=bf)
        for hi, h in enumerate(hs):
            blk = wqT[:, 64 * h:64 * (h + 1)].reshape(CT, 128, 64)
            wq[:, hi, :, 0:64] = blk.transpose(1, 0, 2).astype(bf)
            wq[:, hi, :, 64:128] = wq[:, hi, :, 0:64]
        # kT [128, HPC, MP, 128]: rows 0:64 head-dim of even m-tile,
        # 64:128 of odd m-tile
        kT = np.empty((128, HPC, MP, 128), dtype=bf)
        for hi, h in enumerate(hs):
            kb = k[b, h]
            for p in range(MP):
                kT[0:64, hi, p, :] = kb[256 * p:256 * p + 128, :].T
                kT[64:128, hi, p, :] = kb[256 * p + 128:256 * p + 256, :].T
        # va [128, HPC, MT, 128]: partition = m within tile; cols 0:64 =
        # ones (softmax denominator rows), cols 64:128 = v
        va = np.ones((128, HPC, MT, 128), dtype=bf)
        for hi, h in enumerate(hs):
            va[:, hi, :, D:2 * D] = v[b, h].reshape(MT, 128, D).transpose(
                1, 0, 2).astype(bf)
        # wp [128, HPC, DIM] duplicated on both partition halves
        wp = np.empty((128, HPC, DIM), dtype=bf)
        for hi, h in enumerate(hs):
            wp[0:64, hi, :] = wpT[64 * h:64 * (h + 1), :].astype(bf)
            wp[64:128, hi, :] = wp[0:64, hi, :]
        in_maps.append({"xT": xT, "wq": np.ascontiguousarray(wq),
                        "kT": np.ascontiguousarray(kT),
                        "va": np.ascontiguousarray(va),
                        "wp": np.ascontiguousarray(wp)})
    return in_maps


def kernel(x, k, v, w_qkv, w_proj, b_proj):
    b_proj = np.asarray(b_proj, dtype=np.float32)

    if "nc" not in _cached:
        _cached["nc"] = build_program()
    nc = _cached["nc"]

    in_maps = build_in_maps(x, k, v, w_qkv, w_proj)
    res = run_bass_kernel_spmd(nc, in_maps, core_ids=list(range(NCORES)))

    out = np.empty((B, N, DIM), dtype=np.float32)
    for b in range(B):
        acc = np.zeros((N, DIM), dtype=np.float64)
        for core in range(4 * b, 4 * b + 4):
            acc += res.results[core]["out"].astype(np.float64)
        out[b] = (acc + b_proj).astype(np.float32)
    return out


# revision 25
# speedup vs baseline: 1.0465x; 1.0465x over previous
"""Multi-head attention (GAttention) on 8 trn2 NeuronCores — v2.

Reference computation (per batch b):
    q = x @ w_qkv.T            -> [N, 768], heads of 64
    attn = softmax(q k^T / 8)  -> per head [N, M]
    out_h = attn @ v           -> [N, 64]
    out = concat(out_h) @ w_proj.T + b_proj

Sharding: 24 (b, head) units over 8 cores -> each core one batch b and 3
heads; cores emit f16 partial projections [N, 768]; host sums 4 partials
per batch + bias.

v2 design vs the 190us baseline (all engines rebalanced around the serial
exp softmax cost):
  * exp is split between the Scalar (ACT) engine and a CUSTOM DVE op
    "EXP32Q_ANT": e^x ~ ((t+A)t+B)^32 with t = sigma-scaled logits, a
    single 8-stage Vector-engine instruction (quadratic Horner + 5
    squarings).  sigma = 0.125/(32 kk) is folded into w_q host-side so
    S^T arrives pre-scaled; the ACT path uses activation scale 32 kk.
  * one flat phase: qproj h0 runs first (chunked, 512 cols at a time so
    attention starts ~6us in); qproj h1/h2 and the projection of each
    completed n-quarter are interleaved into the attention iteration
    stream, so the PE never waits on phase barriers.
  * PSUM (8 banks): st 2x[128,2,512] (4) + av 2x[128,512] (2) + one
    shared [128,512]/[128,2,384] ring (2) used by qproj early and proj
    later.
  * proj packs n-tile pairs (rows 0:64 / 64:128 of the PE) into one
    [128,2,384] PSUM tile; a single engine copy casts to f16 and one
    strided DMA writes both n-tiles.
  * outTn row duplication (needed for proj row packing) is done by a
    SBUF->SBUF DMA instead of a second vector multiply.
  * inputs are host-packed partition-major so every DMA is a few large
    contiguous-row transfers (descriptor spreading across all 16 DMA
    engines does the balancing); issue order follows consumption order.
"""
import numpy as np
import ml_dtypes
from contextlib import ExitStack

import concourse.bass as bass
import concourse.mybir as mybir
import concourse.tile as tile
from concourse import bacc
from concourse.bass_utils import run_bass_kernel_spmd

B, N, DIM = 2, 2048, 768
H, D = 12, 64
M = 2048
NCORES = 8
HPC = 3              # heads per core
NT = N // 128        # 16 n tiles
MT = M // 128        # 16 m tiles
MP = MT // 2         # 8 m-tile pairs
CT = DIM // 128      # 6 contraction tiles for qproj
NQ = 4               # n quarters of 512
QW = N // NQ         # 512
F32 = mybir.dt.float32
F16 = mybir.dt.float16
BF16 = mybir.dt.bfloat16

# --- custom DVE exp: e^x ~ ((t + A)*t + B)^32, t = x/(32*KK) -------------
# minimax fit weighted by softmax contribution (see fit, rel err <=1e-3 on
# the weight-bearing logit range; end-to-end rel err 4.9e-3 all-DVE).
KK = 1.33947417
EA = 1.33152807
EB = 1.00021259
ACT_SCALE = 32.0 * KK            # ACT path: e^{st * ACT_SCALE}
SIGMA = 0.125 / (32.0 * KK)      # folded into w_q host-side

import os
DVE_PER16 = int(os.environ.get("KX", "5"))   # exp tiles per 16 on DVE
_DVE_MOD = set(round(i * 16 / max(DVE_PER16, 1) + 1) % 16
               for i in range(DVE_PER16))
LAG = 2

_cached = {}


def _register_exp_op():
    import concourse.dve_ops as dvo
    from concourse.dve_spec import Spec, Src0, C0, C1, sq, lower
    from concourse.dve_uop import DveOpSpec

    name = "EXP32Q_ANT"
    for op in dvo.OPS:
        if op.name == name:
            return op
    p = (Src0 + C0) * Src0 + C1
    for _ in range(5):
        p = sq(p)
    spec = Spec(body=p)
    row = max(dvo._SUB_OPCODE_FOR_NAME.values()) + 1
    assert row < 0x20
    dvo._SUB_OPCODE_FOR_NAME[name] = row
    shas = {}
    for ver in ("v3", "v4"):
        try:
            uops = lower(spec, ver=ver)
            shas[ver] = DveOpSpec(
                name=name, opcode=row, uops=uops, rd1_en=False).sha(ver)
        except Exception:
            pass
    op = dvo.DveOp(name, spec, subdim=False, uops_sha=shas)
    dvo.OPS.append(op)
    dvo.CUSTOM_DVE_SPECS[name] = spec
    return op


def build_program():
    exp_op = _register_exp_op()
    nc = bacc.Bacc("TRN2", target_bir_lowering=False, debug=False)
    xT_d = nc.dram_tensor("xT", [DIM, N], BF16, kind="ExternalInput")
    wq_d = nc.dram_tensor("wq", [128, HPC, CT, 128], BF16,
                          kind="ExternalInput")
    kT_d = nc.dram_tensor("kT", [128, HPC, MP, 128], BF16,
                          kind="ExternalInput")
    va_d = nc.dram_tensor("va", [128, HPC, MT, 128], BF16,
                          kind="ExternalInput")
    wp_d = nc.dram_tensor("wp", [128, HPC, DIM], BF16, kind="ExternalInput")
    out_d = nc.dram_tensor("out", [N, DIM], F32, kind="ExternalOutput")

    with tile.TileContext(nc) as tc, ExitStack() as ctx:
        big = ctx.enter_context(tc.tile_pool(name="big", bufs=1))
        etp = ctx.enter_context(tc.tile_pool(name="etp", bufs=6))
        rsp = ctx.enter_context(tc.tile_pool(name="rsp", bufs=2))
        stg = ctx.enter_context(tc.tile_pool(name="stg", bufs=3))

        # persistent SBUF tensors; DMA issue order == consumption order,
        # spread across engine queues (each dma_start costs ~600ns of issue
        # time on its sequencer, so the head-of-kernel loads go wide).
        wq_t = big.tile([128, HPC, CT, 128], BF16)
        nc.scalar.dma_start(wq_t[:], wq_d[:])
        xT_t = [big.tile([128, N], BF16, name=f"xT{c}", tag=f"xT{c}")
                for c in range(CT)]
        for c in range(CT):
            eng = nc.sync if c % 2 == 0 else nc.scalar
            eng.dma_start(xT_t[c][:, 0:QW],
                          xT_d[c * 128:(c + 1) * 128, 0:QW])
        kT_t = big.tile([128, HPC, MP, 128], BF16)
        nc.sync.dma_start(kT_t[:, :, 0:2, :], kT_d[:, :, 0:2, :])
        va_t = big.tile([128, HPC, MT, 128], BF16)
        nc.sync.dma_start(va_t[:, :, 0:4, :], va_d[:, :, 0:4, :])
        nc.sync.dma_start(kT_t[:, :, 2:MP, :], kT_d[:, :, 2:MP, :])
        nc.sync.dma_start(va_t[:, :, 4:MT, :], va_d[:, :, 4:MT, :])
        for c in range(CT):
            nc.sync.dma_start(xT_t[c][:, QW:N],
                              xT_d[c * 128:(c + 1) * 128, QW:N])
        wp_t = big.tile([128, HPC, DIM], BF16)
        nc.sync.dma_start(wp_t[:], wp_d[:])

        qT_t = big.tile([128, HPC, N], BF16)
        outTn_t = big.tile([128, HPC, N], BF16)
        zz_t = big.tile([128, 64], BF16)
        nc.vector.memset(zz_t[:], 0.0)

        st_ps = ctx.enter_context(
            tc.tile_pool(name="st_ps", bufs=2, space="PSUM"))
        acc_ps = ctx.enter_context(
            tc.tile_pool(name="acc_ps", bufs=2, space="PSUM"))

        def qproj_chunk(h, ch):
            # q columns [ch*512, (ch+1)*512) for head h (duplicated rows)
            qp = acc_ps.tile([128, 2, QW], F32, tag="acc", name="acc",
                             bufs=1)
            nsl = slice(ch * QW, (ch + 1) * QW)
            for c in range(CT):
                nc.tensor.matmul(
                    qp[:, 0, :], wq_t[:, h, c, :], xT_t[c][:, nsl],
                    start=(c == 0), stop=(c == CT - 1),
                )
            nc.vector.tensor_copy(qT_t[:, h, nsl], qp[:, 0, :])

        def proj_group(q, j, oc):
            # n-tile pair (a=rows 0:64 lhs, b=rows 64:128), output column
            # slice oc*384; one packed PSUM tile, one f16 copy, one DMA.
            na = (q * 4 + 2 * j) * 128
            nb = na + 128
            osl = slice(oc * 384, (oc + 1) * 384)
            pj = acc_ps.tile([128, 2, QW], F32, tag="acc", name="acc",
                             bufs=1)
            pjv = pj[:, :, 0:384]
            for hh in range(HPC):
                nc.tensor.matmul(
                    pjv[:, 0, :], outTn_t[0:64, hh, na:na + 128],
                    wp_t[0:64, hh, osl],
                    start=(hh == 0), stop=(hh == HPC - 1),
                    tile_position=(0, 0),
                )
                nc.tensor.matmul(
                    pjv[:, 1, :], outTn_t[64:128, hh, nb:nb + 128],
                    wp_t[64:128, hh, osl],
                    start=(hh == 0), stop=(hh == HPC - 1),
                    tile_position=(64, 0),
                )
            ot = stg.tile([128, 2, 384], F32, tag="ot", name="ot")
            if (j + oc) % 2 == 0:
                nc.scalar.copy(ot[:], pjv[:])
            else:
                nc.vector.tensor_copy(ot[:], pjv[:])
            nc.sync.dma_start(out_d[na:na + 128, osl], ot[:, 0, :])
            nc.sync.dma_start(out_d[nb:nb + 128, osl], ot[:, 1, :])

        # attention iteration stream with injected qproj/proj work; unit
        # (0,h1) starts at gi 8 and quarter 1 at gi 24, so chunk (h,ch) is
        # always emitted well before its first reader
        units = [(q, h) for q in range(NQ) for h in range(HPC)]
        # ch0 of h1/h2 depends only on the early xT quarter; ch>=1 chunks
        # wait for the xT tail DMA (~25us in), so inject them just before
        # their quarter-1 consumption at gi 24
        inject = {2: [("qproj", 1, 0)], 5: [("qproj", 2, 0)]}
        for ch in range(1, 4):
            for h in range(HPC):
                inject[10 + 3 * ch + h] = [("qproj", h, ch)]
        # proj for quarter q injects only after unit (q, h2)'s lagged AV
        # flush + norm have been EMITTED (at iter base+1's flush), else the
        # dep tracker can't order the proj reads after the norm writes.
        for q in range(NQ - 1):
            base = (3 * q + 3) * 8 + 1
            for g in range(4):
                inject.setdefault(base + 2 * g, []).append(
                    ("proj", q, g // 2, g % 2))

        av_by_unit = {}
        pend = []

        def _av(pd):
            (q, h), et, p, first, last, unit_last = pd
            av = av_by_unit[(q, h)]
            nc.tensor.matmul(av[:], va_t[:, h, 2 * p, :], et[:, 0, :],
                             start=first, stop=False)
            nc.tensor.matmul(av[:], va_t[:, h, 2 * p + 1, :], et[:, 1, :],
                             start=False, stop=last)

        def _norm(q, h):
            av = av_by_unit[(q, h)]
            nsl = slice(q * QW, (q + 1) * QW)
            rs = rsp.tile([64, QW], F32, tag="rs", name="rs")
            # va cols 0:64 are ones -> av[0:64] = denominator (base-0 AP:
            # custom-DVE ops misread partition-base-64 inputs)
            nc.vector.reciprocal_approx_fast(rs[:], av[0:64, :])
            nc.vector.tensor_mul(outTn_t[0:64, h, nsl], av[64:128, :], rs[:])
            nc.sync.dma_start(outTn_t[64:128, h, nsl],
                              outTn_t[0:64, h, nsl])

        def _flush(limit):
            while len(pend) > limit:
                pd = pend.pop(0)
                _av(pd)
                if pd[5]:
                    _norm(*pd[0])

        # PE p-state warmup: dummy matmuls on a zeroed tile into scrap PSUM
        # keep the array busy while the first DMAs land
        for w in range(2):
            wt = st_ps.tile([128, 2, QW], F32, tag="st", name="st")
            for i in range(8):
                nc.tensor.matmul(wt[0:64, 0, 0:64], zz_t[:, 0:64],
                                 zz_t[:, 0:64], start=True, stop=True)

        qproj_chunk(0, 0)

        gi = 0
        for (q, h) in units:
            nsl = slice(q * QW, (q + 1) * QW)
            for p in range(MP):
                if p == 0:
                    av_by_unit[(q, h)] = acc_ps.tile(
                        [128, QW], F32, tag="av", name="av")
                st = st_ps.tile([128, 2, QW], F32, tag="st", name="st")
                nc.tensor.matmul(
                    st[:, 0, :], kT_t[0:64, h, p, :], qT_t[0:64, h, nsl],
                    start=True, stop=True, tile_position=(0, 0),
                )
                nc.tensor.matmul(
                    st[:, 1, :], kT_t[64:128, h, p, :],
                    qT_t[64:128, h, nsl],
                    start=True, stop=True, tile_position=(64, 0),
                )
                _flush(LAG - 1)
                et = etp.tile([128, 2, QW], BF16, tag="et", name="et")
                if (gi % 16) in _DVE_MOD:
                    nc.vector._custom_dve(
                        exp_op, out=et[:], in0=st[:],
                        s0=float(EA), s1=float(EB))
                else:
                    nc.scalar.activation(
                        et[:], st[:], mybir.ActivationFunctionType.Exp,
                        scale=ACT_SCALE)
                pend.append(((q, h), et, p, p == 0, p == MP - 1,
                             p == MP - 1))
                for item in inject.get(gi, ()):
                    if item[0] == "qproj":
                        qproj_chunk(item[1], item[2])
                    else:
                        proj_group(item[1], item[2], item[3])
                gi += 1
        _flush(0)
        for g in range(4):
            proj_group(NQ - 1, g // 2, g % 2)

    nc.compile()
    return nc


def build_in_maps(x, k, v, w_qkv, w_proj):
    x = np.asarray(x, dtype=np.float32)
    k = np.asarray(k, dtype=np.float32)
    v = np.asarray(v, dtype=np.float32)
    wqT = np.ascontiguousarray(np.asarray(w_qkv, np.float32).T) * SIGMA
    wpT = np.ascontiguousarray(np.asarray(w_proj, np.float32).T)
    bf = ml_dtypes.bfloat16

    in_maps = []
    for core in range(NCORES):
        b = core // 4
        hs = [3 * (core % 4) + i for i in range(HPC)]
        xT = np.ascontiguousarray(x[b].T.astype(bf))
        # wq packed [128, HPC, CT, 128]: partition = contraction row within
        # c-tile; last dim = duplicated 64-wide head slice
        wq = np.empty((128, HPC, CT, 128), dtype=bf)
        for hi, h in enumerate(hs):
            blk = wqT[:, 64 * h:64 * (h + 1)].reshape(CT, 128, 64)
            wq[:, hi, :, 0:64] = blk.transpose(1, 0, 2).astype(bf)
            wq[:, hi, :, 64:128] = wq[:, hi, :, 0:64]
        # kT [128, HPC, MP, 128]: rows 0:64 head-dim of even m-tile,
        # 64:128 of odd m-tile
        kT = np.empty((128, HPC, MP, 128), dtype=bf)
        for hi, h in enumerate(hs):
            kb = k[b, h]
            for p in range(MP):
                kT[0:64, hi, p, :] = kb[256 * p:256 * p + 128, :].T
                kT[64:128, hi, p, :] = kb[256 * p + 128:256 * p + 256, :].T
        # va [128, HPC, MT, 128]: partition = m within tile; cols 0:64 =
        # ones (softmax denominator rows), cols 64:128 = v
        va = np.ones((128, HPC, MT, 128), dtype=bf)
        for hi, h in enumerate(hs):
            va[:, hi, :, D:2 * D] = v[b, h].reshape(MT, 128, D).transpose(
                1, 0, 2).astype(bf)
        # wp [128, HPC, DIM] duplicated on both partition halves
        wp = np.empty((128, HPC, DIM), dtype=bf)
        for hi, h in enumerate(hs):
            wp[0:64, hi, :] = wpT[64 * h:64 * (h + 1), :].astype(bf)
            wp[64:128, hi, :] = wp[0:64, hi, :]
        in_maps.append({"xT": xT, "wq": np.ascontiguousarray(wq),
                        "kT": np.ascontiguousarray(kT),
                        "va": np.ascontiguousarray(va),
                        "wp": np.ascontiguousarray(wp)})
    return in_maps


def kernel(x, k, v, w_qkv, w_proj, b_proj):
    b_proj = np.asarray(b_proj, dtype=np.float32)

    if "nc" not in _cached:
        _cached["nc"] = build_program()
    nc = _cached["nc"]

    in_maps = build_in_maps(x, k, v, w_qkv, w_proj)
    res = run_bass_kernel_spmd(nc, in_maps, core_ids=list(range(NCORES)))

    out = np.empty((B, N, DIM), dtype=np.float32)
    for b in range(B):
        acc = np.zeros((N, DIM), dtype=np.float64)
        for core in range(4 * b, 4 * b + 4):
            acc += res.results[core]["out"].astype(np.float64)
        out[b] = (acc + b_proj).astype(np.float32)
    return out


# revision 31
# speedup vs baseline: 1.0894x; 1.0410x over previous
"""Multi-head attention (GAttention) on 8 trn2 NeuronCores — v2.

Reference computation (per batch b):
    q = x @ w_qkv.T            -> [N, 768], heads of 64
    attn = softmax(q k^T / 8)  -> per head [N, M]
    out_h = attn @ v           -> [N, 64]
    out = concat(out_h) @ w_proj.T + b_proj

Sharding: 24 (b, head) units over 8 cores -> each core one batch b and 3
heads; cores emit f16 partial projections [N, 768]; host sums 4 partials
per batch + bias.

v2 design vs the 190us baseline (all engines rebalanced around the serial
exp softmax cost):
  * exp is split between the Scalar (ACT) engine and a CUSTOM DVE op
    "EXP32Q_ANT": e^x ~ ((t+A)t+B)^32 with t = sigma-scaled logits, a
    single 8-stage Vector-engine instruction (quadratic Horner + 5
    squarings).  sigma = 0.125/(32 kk) is folded into w_q host-side so
    S^T arrives pre-scaled; the ACT path uses activation scale 32 kk.
  * one flat phase: qproj h0 runs first (chunked, 512 cols at a time so
    attention starts ~6us in); qproj h1/h2 and the projection of each
    completed n-quarter are interleaved into the attention iteration
    stream, so the PE never waits on phase barriers.
  * PSUM (8 banks): st 2x[128,2,512] (4) + av 2x[128,512] (2) + one
    shared [128,512]/[128,2,384] ring (2) used by qproj early and proj
    later.
  * proj packs n-tile pairs (rows 0:64 / 64:128 of the PE) into one
    [128,2,384] PSUM tile; a single engine copy casts to f16 and one
    strided DMA writes both n-tiles.
  * outTn row duplication (needed for proj row packing) is done by a
    SBUF->SBUF DMA instead of a second vector multiply.
  * inputs are host-packed partition-major so every DMA is a few large
    contiguous-row transfers (descriptor spreading across all 16 DMA
    engines does the balancing); issue order follows consumption order.
"""
import numpy as np
import ml_dtypes
from contextlib import ExitStack

import concourse.bass as bass
import concourse.mybir as mybir
import concourse.tile as tile
from concourse import bacc
from concourse.bass_utils import run_bass_kernel_spmd

B, N, DIM = 2, 2048, 768
H, D = 12, 64
M = 2048
NCORES = 8
HPC = 3              # heads per core
NT = N // 128        # 16 n tiles
MT = M // 128        # 16 m tiles
MP = MT // 2         # 8 m-tile pairs
CT = DIM // 128      # 6 contraction tiles for qproj
NQ = 4               # n quarters of 512
QW = N // NQ         # 512
F32 = mybir.dt.float32
F16 = mybir.dt.float16
BF16 = mybir.dt.bfloat16

# --- custom DVE exp: e^x ~ ((t + A)*t + B)^32, t = x/(32*KK) -------------
# minimax fit weighted by softmax contribution (see fit, rel err <=1e-3 on
# the weight-bearing logit range; end-to-end rel err 4.9e-3 all-DVE).
KK = 1.33947417
EA = 1.33152807
EB = 1.00021259
ACT_SCALE = 32.0 * KK            # ACT path: e^{st * ACT_SCALE}
SIGMA = 0.125 / (32.0 * KK)      # folded into w_q host-side

import os
DVE_PER16 = int(os.environ.get("KX", "5"))   # exp tiles per 16 on DVE
_DVE_MOD = set(round(i * 16 / max(DVE_PER16, 1) + 1) % 16
               for i in range(DVE_PER16))
LAG = 2

_cached = {}


def _register_exp_op():
    import concourse.dve_ops as dvo
    from concourse.dve_spec import Spec, Src0, C0, C1, sq, lower
    from concourse.dve_uop import DveOpSpec

    name = "EXP32Q_ANT"
    for op in dvo.OPS:
        if op.name == name:
            return op
    p = (Src0 + C0) * Src0 + C1
    for _ in range(5):
        p = sq(p)
    spec = Spec(body=p)
    row = max(dvo._SUB_OPCODE_FOR_NAME.values()) + 1
    assert row < 0x20
    dvo._SUB_OPCODE_FOR_NAME[name] = row
    shas = {}
    for ver in ("v3", "v4"):
        try:
            uops = lower(spec, ver=ver)
            shas[ver] = DveOpSpec(
                name=name, opcode=row, uops=uops, rd1_en=False).sha(ver)
        except Exception:
            pass
    op = dvo.DveOp(name, spec, subdim=False, uops_sha=shas)
    dvo.OPS.append(op)
    dvo.CUSTOM_DVE_SPECS[name] = spec
    return op


def build_program():
    exp_op = _register_exp_op()
    nc = bacc.Bacc("TRN2", target_bir_lowering=False, debug=False)
    xT_d = nc.dram_tensor("xT", [DIM, N], BF16, kind="ExternalInput")
    wq_d = nc.dram_tensor("wq", [128, HPC, CT, 128], BF16,
                          kind="ExternalInput")
    kT_d = nc.dram_tensor("kT", [128, HPC, MP, 128], BF16,
                          kind="ExternalInput")
    va_d = nc.dram_tensor("va", [128, HPC, MT, 128], BF16,
                          kind="ExternalInput")
    wp_d = nc.dram_tensor("wp", [128, HPC, DIM], BF16, kind="ExternalInput")
    out_d = nc.dram_tensor("out", [N, DIM], F32, kind="ExternalOutput")

    with tile.TileContext(nc) as tc, ExitStack() as ctx:
        big = ctx.enter_context(tc.tile_pool(name="big", bufs=1))
        etp = ctx.enter_context(tc.tile_pool(name="etp", bufs=6))
        rsp = ctx.enter_context(tc.tile_pool(name="rsp", bufs=2))
        stg = ctx.enter_context(tc.tile_pool(name="stg", bufs=3))

        # persistent SBUF tensors; DMA issue order == consumption order,
        # spread across engine queues (each dma_start costs ~600ns of issue
        # time on its sequencer, so the head-of-kernel loads go wide).
        wq_t = big.tile([128, HPC, CT, 128], BF16)
        nc.scalar.dma_start(wq_t[:, 0:1, :, :], wq_d[:, 0:1, :, :])
        kT_t = big.tile([128, HPC, MP, 128], BF16)
        nc.sync.dma_start(kT_t[:, :, 0:2, :], kT_d[:, :, 0:2, :])
        xT_t = [big.tile([128, N], BF16, name=f"xT{c}", tag=f"xT{c}")
                for c in range(CT)]
        for c in range(CT):
            eng = nc.sync if c % 2 == 0 else nc.scalar
            eng.dma_start(xT_t[c][:, 0:QW],
                          xT_d[c * 128:(c + 1) * 128, 0:QW])
        va_t = big.tile([128, HPC, MT, 128], BF16)
        nc.sync.dma_start(va_t[:, :, 0:4, :], va_d[:, :, 0:4, :])
        nc.scalar.dma_start(wq_t[:, 1:HPC, :, :], wq_d[:, 1:HPC, :, :])
        nc.sync.dma_start(kT_t[:, :, 2:MP, :], kT_d[:, :, 2:MP, :])
        nc.sync.dma_start(va_t[:, :, 4:MT, :], va_d[:, :, 4:MT, :])
        for c in range(CT):
            nc.sync.dma_start(xT_t[c][:, QW:N],
                              xT_d[c * 128:(c + 1) * 128, QW:N])
        wp_t = big.tile([128, HPC, DIM], BF16)
        nc.sync.dma_start(wp_t[:], wp_d[:])

        qT_t = big.tile([128, HPC, N], BF16)
        outTn_t = big.tile([128, HPC, N], BF16)
        zz_t = big.tile([128, 64], BF16)
        nc.vector.memset(zz_t[:], 0.0)

        acc_ps = ctx.enter_context(
            tc.tile_pool(name="acc_ps", bufs=2, space="PSUM"))
        st_stack = ExitStack()
        st_ps = st_stack.enter_context(
            tc.tile_pool(name="st_ps", bufs=2, space="PSUM"))

        # injected PE work comes in small pieces (<=3 matmuls per iter) so
        # the S^T->EXP stream never stalls behind a long injected burst
        qp_live = {}

        def qp_piece(h, ch, second):
            nsl = slice(ch * QW, (ch + 1) * QW)
            if not second:
                qp_live[(h, ch)] = acc_ps.tile(
                    [128, 2, QW], F32, tag="acc", name="acc", bufs=1)
            qp = qp_live[(h, ch)]
            for c in (range(3, CT) if second else range(3)):
                nc.tensor.matmul(
                    qp[:, 0, :], wq_t[:, h, c, :], xT_t[c][:, nsl],
                    start=(c == 0), stop=(c == CT - 1),
                )
            if second:
                nc.vector.tensor_copy(qT_t[:, h, nsl], qp[:, 0, :])
                del qp_live[(h, ch)]

        pj_live = {}

        def pj_piece(q, j, oc, second, pool, nbufs=1):
            # n-tile pair (a=rows 0:64 lhs, b=rows 64:128), output column
            # slice oc*384; one packed PSUM tile, one copy, two DMAs
            na = (q * 4 + 2 * j) * 128
            nb = na + 128
            osl = slice(oc * 384, (oc + 1) * 384)
            if not second:
                pj_live[(q, j, oc)] = pool.tile(
                    [128, 2, QW], F32, tag="acc", name="acc", bufs=nbufs)
            pjv = pj_live[(q, j, oc)][:, :, 0:384]
            for hh in ((2,) if second else (0, 1)):
                nc.tensor.matmul(
                    pjv[:, 0, :], outTn_t[0:64, hh, na:na + 128],
                    wp_t[0:64, hh, osl],
                    start=(hh == 0), stop=(hh == HPC - 1),
                    tile_position=(0, 0),
                )
                nc.tensor.matmul(
                    pjv[:, 1, :], outTn_t[64:128, hh, nb:nb + 128],
                    wp_t[64:128, hh, osl],
                    start=(hh == 0), stop=(hh == HPC - 1),
                    tile_position=(64, 0),
                )
            if second:
                ot = stg.tile([128, 2, 384], F32, tag="ot", name="ot")
                if (j + oc) % 2 == 0:
                    nc.scalar.copy(ot[:], pjv[:])
                else:
                    nc.vector.tensor_copy(ot[:], pjv[:])
                nc.sync.dma_start(out_d[na:na + 128, osl], ot[:, 0, :])
                nc.sync.dma_start(out_d[nb:nb + 128, osl], ot[:, 1, :])
                del pj_live[(q, j, oc)]

        units = [(q, h) for q in range(NQ) for h in range(HPC)]
        # schedule: gi -> piece; qproj ch0 early (needs only the xT head
        # quarter), ch>=1 after the xT tail lands (~gi 13); proj quarter q
        # only after unit (q,h2)'s lagged flush+norm are EMITTED (base+1)
        inject = {}

        def sched_qp(h, ch, g0):
            inject[g0] = ("qp", h, ch, False)
            inject[g0 + 1] = ("qp", h, ch, True)

        sched_qp(1, 0, 2)
        sched_qp(2, 0, 5)
        for ch in range(1, 4):
            for h in range(HPC):
                g0 = {1: 13, 2: 33, 3: 57}[ch] + 2 * h
                sched_qp(h, ch, g0)
        for q in range(NQ - 1):
            base = (3 * q + 3) * 8 + 1
            for g in range(4):
                j, oc = g // 2, g % 2
                inject[base + 2 * g] = ("pj", q, j, oc, False)
                inject[base + 2 * g + 1] = ("pj", q, j, oc, True)

        av_by_unit = {}
        pend = []

        def _av(pd):
            (q, h), et, p, first, last, unit_last = pd
            av = av_by_unit[(q, h)]
            nc.tensor.matmul(av[:], va_t[:, h, 2 * p, :], et[:, 0, :],
                             start=first, stop=False)
            nc.tensor.matmul(av[:], va_t[:, h, 2 * p + 1, :], et[:, 1, :],
                             start=False, stop=last)

        def _norm(q, h):
            av = av_by_unit[(q, h)]
            nsl = slice(q * QW, (q + 1) * QW)
            rs = rsp.tile([64, QW], F32, tag="rs", name="rs")
            # va cols 0:64 are ones -> av[0:64] = denominator (base-0 AP:
            # custom-DVE ops misread partition-base-64 inputs)
            nc.vector.reciprocal_approx_fast(rs[:], av[0:64, :])
            nc.vector.tensor_mul(outTn_t[0:64, h, nsl], av[64:128, :], rs[:])
            nc.sync.dma_start(outTn_t[64:128, h, nsl],
                              outTn_t[0:64, h, nsl])

        def _flush(limit):
            while len(pend) > limit:
                pd = pend.pop(0)
                _av(pd)
                if pd[5]:
                    _norm(*pd[0])

        # PE p-state warmup: dummy matmuls on a zeroed tile into scrap PSUM
        # keep the array busy while the first DMAs land
        for w in range(2):
            wt = st_ps.tile([128, 2, QW], F32, tag="st", name="st")
            for i in range(8):
                nc.tensor.matmul(wt[0:64, 0, 0:64], zz_t[:, 0:64],
                                 zz_t[:, 0:64], start=True, stop=True)

        qp_piece(0, 0, False)
        qp_piece(0, 0, True)

        gi = 0
        for (q, h) in units:
            nsl = slice(q * QW, (q + 1) * QW)
            for p in range(MP):
                if p == 0:
                    av_by_unit[(q, h)] = acc_ps.tile(
                        [128, QW], F32, tag="av", name="av")
                st = st_ps.tile([128, 2, QW], F32, tag="st", name="st")
                nc.tensor.matmul(
                    st[:, 0, :], kT_t[0:64, h, p, :], qT_t[0:64, h, nsl],
                    start=True, stop=True, tile_position=(0, 0),
                )
                nc.tensor.matmul(
                    st[:, 1, :], kT_t[64:128, h, p, :],
                    qT_t[64:128, h, nsl],
                    start=True, stop=True, tile_position=(64, 0),
                )
                _flush(LAG - 1)
                et = etp.tile([128, 2, QW], BF16, tag="et", name="et")
                if (gi % 16) in _DVE_MOD:
                    nc.vector._custom_dve(
                        exp_op, out=et[:], in0=st[:],
                        s0=float(EA), s1=float(EB))
                else:
                    nc.scalar.activation(
                        et[:], st[:], mybir.ActivationFunctionType.Exp,
                        scale=ACT_SCALE)
                pend.append(((q, h), et, p, p == 0, p == MP - 1,
                             p == MP - 1))
                item = inject.get(gi)
                if item is not None:
                    if item[0] == "qp":
                        qp_piece(item[1], item[2], item[3])
                    else:
                        pj_piece(item[1], item[2], item[3], item[4],
                                 acc_ps)
                gi += 1
        _flush(0)
        st_stack.close()
        # tail: the freed st banks give the last quarter's proj a
        # double-buffered ring so fills overlap copies
        with tc.tile_pool(name="tail_ps", bufs=2, space="PSUM") as tailp:
            for jj in range(2):
                pj_piece(NQ - 1, jj, 0, False, tailp, 2)
                pj_piece(NQ - 1, jj, 1, False, tailp, 2)
                pj_piece(NQ - 1, jj, 0, True, tailp, 2)
                pj_piece(NQ - 1, jj, 1, True, tailp, 2)

    nc.compile()
    return nc


def build_in_maps(x, k, v, w_qkv, w_proj):
    x = np.asarray(x, dtype=np.float32)
    k = np.asarray(k, dtype=np.float32)
    v = np.asarray(v, dtype=np.float32)
    wqT = np.ascontiguousarray(np.asarray(w_qkv, np.float32).T) * SIGMA
    wpT = np.ascontiguousarray(np.asarray(w_proj, np.float32).T)
    bf = ml_dtypes.bfloat16

    in_maps = []
    for core in range(NCORES):
        b = core // 4
        hs = [3 * (core % 4) + i for i in range(HPC)]
        xT = np.ascontiguousarray(x[b].T.astype(bf))
        # wq packed [128, HPC, CT, 128]: partition = contraction row within
        # c-tile; last dim = duplicated 64-wide head slice
        wq = np.empty((128, HPC, CT, 128), dtype=bf)
        for hi, h in enumerate(hs):
            blk = wqT[:, 64 * h:64 * (h + 1)].reshape(CT, 128, 64)
            wq[:, hi, :, 0:64] = blk.transpose(1, 0, 2).astype(bf)
            wq[:, hi, :, 64:128] = wq[:, hi, :, 0:64]
        # kT [128, HPC, MP, 128]: rows 0:64 head-dim of even m-tile,
        # 64:128 of odd m-tile
        kT = np.empty((128, HPC, MP, 128), dtype=bf)
        for hi, h in enumerate(hs):
            kb = k[b, h]
            for p in range(MP):
                kT[0:64, hi, p, :] = kb[256 * p:256 * p + 128, :].T
                kT[64:128, hi, p, :] = kb[256 * p + 128:256 * p + 256, :].T
        # va [128, HPC, MT, 128]: partition = m within tile; cols 0:64 =
        # ones (softmax denominator rows), cols 64:128 = v
        va = np.ones((128, HPC, MT, 128), dtype=bf)
        for hi, h in enumerate(hs):
            va[:, hi, :, D:2 * D] = v[b, h].reshape(MT, 128, D).transpose(
                1, 0, 2).astype(bf)
        # wp [128, HPC, DIM] duplicated on both partition halves
        wp = np.empty((128, HPC, DIM), dtype=bf)
        for hi, h in enumerate(hs):
            wp[0:64, hi, :] = wpT[64 * h:64 * (h + 1), :].astype(bf)
            wp[64:128, hi, :] = wp[0:64, hi, :]
        in_maps.append({"xT": xT, "wq": np.ascontiguousarray(wq),
                        "kT": np.ascontiguousarray(kT),
                        "va": np.ascontiguousarray(va),
                        "wp": np.ascontiguousarray(wp)})
    return in_maps


def kernel(x, k, v, w_qkv, w_proj, b_proj):
    b_proj = np.asarray(b_proj, dtype=np.float32)

    if "nc" not in _cached:
        _cached["nc"] = build_program()
    nc = _cached["nc"]

    in_maps = build_in_maps(x, k, v, w_qkv, w_proj)
    res = run_bass_kernel_spmd(nc, in_maps, core_ids=list(range(NCORES)))

    out = np.empty((B, N, DIM), dtype=np.float32)
    for b in range(B):
        acc = np.zeros((N, DIM), dtype=np.float64)
        for core in range(4 * b, 4 * b + 4):
            acc += res.results[core]["out"].astype(np.float64)
        out[b] = (acc + b_proj).astype(np.float32)
    return out
